# revision 12
# baseline (speedup 1.0000x reference)
"""Trainium2 Bass kernel for batched 1D max-plus dilation with parabolic
structuring element:

    out[b, i] = max_{|d| <= 100, 0 <= i+d < L} ( x[b, i+d] + h[d+100] ),
    h = -linspace(-100,100,201)^2 / (4*scale)

Strategy
--------
- Pure data parallel: shard B=131072 rows across 8 NeuronCores (16384 each).
- Tap pruning: a tap at displacement d can only win the max if
  x[b, i+d] + h(d) > x[b, i]  (the d=0 candidate), which requires
  d^2 < ~4 s * (max(x) - min(x)).  The radius is computed from the actual
  data, so the result is exactly equal to the full-window reference.
- h is computed with jax.numpy on CPU exactly like the reference
  (jnp.linspace in fp32 is NOT exact integers).
- Each 128-partition tile holds R=32 rows per partition, each row in a
  (201+PAD)-word slot (PAD-word -1e30 pad shared between consecutive rows),
  so a single strided AP covers all rows for a given tap shift.
- Per tap: one fused DVE instruction
      acc = (x_shift + h_d) max acc        (scalar_tensor_tensor)
  fp32 end-to-end => same rounding as the fp32 jax reference.
"""

import math
import os
import sys

import numpy as np

for _p in ("/opt/trn_rl_repo", "/root/.axon_site/_ro/trn_rl_repo"):
    if os.path.isdir(_p) and _p not in sys.path:
        sys.path.insert(0, _p)

L = 201          # row length (fixed domain in the source model)
K_FULL = 201     # full window size in the source model
N_CORES = 8
R = 64           # rows per partition per tile

# test.py introspection: last run's BassKernelResults per call
LAST_RESULTS = None


def _h_table(scale: float) -> np.ndarray:
    """h[j], j = d+100, computed exactly as the fp32 jax reference does."""
    import jax
    import jax.numpy as jnp

    cpu = jax.devices("cpu")[0]
    with jax.default_device(cpu):
        z = jnp.linspace(-100.0, 100.0, K_FULL, dtype=jnp.float32) ** 2
        h = -z / (jnp.float32(4.0) * jnp.float32(scale))
        return np.asarray(h, dtype=np.float32)


def _pick_radius(x: np.ndarray, scale: float, h: np.ndarray) -> int:
    """Smallest r such that taps with |d| > r provably never win the max:
    candidate(d) <= xmax + h(d) <= xmin <= answer for all dropped d."""
    xmax = float(x.max())
    xmin = float(x.min())
    r = 1
    for d in range(100, 1, -1):
        hv = min(float(h[100 + d]), float(h[100 - d]))
        if xmax + hv > xmin - 1e-3:  # margin
            r = d
            break
    return min(max(r, 1), 100)


_DRAIN_PATCHED = False


def _patch_chunked_tail_drain():
    """The walrus build in this container allows only a small number of sem
    waits per instruction; Tile's kernel-tail drain carries one wait per
    used semaphore lane (engine sems + DMA lanes) on a single Drain, which
    gets rejected. Split the waits across a chain of single-wait drains."""
    global _DRAIN_PATCHED
    if _DRAIN_PATCHED:
        return
    _DRAIN_PATCHED = True

    import concourse.mybir as mybir
    from concourse import tile
    from concourse.vector_clock import ScopedClock

    def _drain_and_barrier(self, tick_clock, wait_clock):
        drain_inst = self.nc.sync.drain()
        wait_clock.add_sem_waits(
            drain_inst.ins, ScopedClock({None: tick_clock.global_clock})
        )
        si = drain_inst.ins.sync_info
        waits = list(si.on_wait or []) if si else []
        if len(waits) > 1:
            drain_inst.ins.sync_info = mybir.SyncInfo(
                on_wait=waits[:1], on_update=[])
            for w in waits[1:]:
                extra = self.nc.sync.drain()
                extra.ins.sync_info = mybir.SyncInfo(
                    on_wait=[w], on_update=[])

        self.nc.all_engine_barrier()
        assert self.sems is not None
        popped = self.nc._tile_sem_poison_stack.pop()
        assert popped is self._sem_poison
        self.nc.clear_and_free_semaphores(list(self.sems.allocated().values()))
        self.nc.all_engine_barrier()

    tile.TileContext._drain_and_barrier = _drain_and_barrier


def _build_program(rows: int, r: int, h: np.ndarray, repeat: int = 1,
                   split_gpsimd: int = 0):
    """Bass program computing the dilation for `rows` rows on one core.

    No padding: each tap d only updates its valid output columns
    [max(0,-d), L-max(0,d)), which reproduces the reference's -inf
    boundary semantics exactly. Rows are packed contiguously.
    """
    import concourse.bass as bass
    import concourse.mybir as mybir
    from concourse.tile import TileContext

    _patch_chunked_tail_drain()

    f32 = mybir.dt.float32
    add = mybir.AluOpType.add
    mx = mybir.AluOpType.max

    assert rows % (128 * R) == 0
    T = rows // (128 * R)

    nc = bass.Bass()
    x = nc.dram_tensor("x", [rows, L], f32, kind="ExternalInput")
    out = nc.dram_tensor("out", [rows, L], f32, kind="ExternalOutput")

    # remaining taps after the first fused (+1, 0) instruction
    ds = [-1] + [sd * d for d in range(2, r + 1) for sd in (1, -1)]

    def hv(d):
        return float(h[100 + d])

    # walrus in this container allows few sem waits per instruction and the
    # kernel-tail drain waits on every used DMA sem lane, so keep the total
    # number of DMA instructions small: T=2 tiles x (1 in + 1 out) = 4 lanes.
    # `repeat` (timing mode) reruns only the compute chain, no extra DMAs.
    with TileContext(nc) as tc:
        with (
            tc.tile_pool(name="xp", bufs=2) as xp,
            tc.tile_pool(name="accp", bufs=2) as accp,
        ):
            for t in range(T):
                xf = xp.tile([128, R * L], f32, name="xf")
                acc = accp.tile([128, R * L], f32, name="acc")

                src = x[t * 128 * R:(t + 1) * 128 * R, :].rearrange(
                    "(p s) c -> p (s c)", s=R)
                nc.sync.dma_start(xf[:, :], src)

                x3 = xf.rearrange("p (s c) -> p s c", c=L)
                acc3 = acc.rearrange("p (s c) -> p s c", c=L)

                splits = []  # (engine, slot_lo, slot_hi)
                if split_gpsimd > 0:
                    splits.append((nc.vector, 0, R - split_gpsimd))
                    splits.append((nc.gpsimd, R - split_gpsimd, R))
                else:
                    splits.append((nc.vector, 0, R))

                for rep in range(repeat):
                    for eng, lo, hi in splits:
                        # walrus allows only ONE sem wait per compute
                        # instruction: this memset takes the WAR wait
                        # (prev DMA-out reading the acc slot), the copy
                        # takes the RAW wait (DMA-in), later ops need none
                        eng.memset(acc3[:, lo:hi, 0:1], 0.0)
                        # last column: only tap 0 of {+1, 0} applies
                        eng.tensor_copy(acc3[:, lo:hi, L - 1:L],
                                        x3[:, lo:hi, L - 1:L])
                        # fused taps (+1, 0) over columns [0, L-1)
                        eng.scalar_tensor_tensor(
                            acc3[:, lo:hi, 0:L - 1],
                            x3[:, lo:hi, 1:L], hv(1),
                            x3[:, lo:hi, 0:L - 1], add, mx)
                        for d in ds:
                            a, b = max(0, -d), L - max(0, d)
                            eng.scalar_tensor_tensor(
                                acc3[:, lo:hi, a:b],
                                x3[:, lo:hi, a + d:b + d], hv(d),
                                acc3[:, lo:hi, a:b], add, mx)

                dst = out[t * 128 * R:(t + 1) * 128 * R, :].rearrange(
                    "(p s) c -> p (s c)", s=R)
                nc.sync.dma_start(dst, acc[:, :])

    return nc


def kernel(x: np.ndarray, scale: np.ndarray, _repeat: int = 1,
           _split_gpsimd: int = 0) -> np.ndarray:
    global LAST_RESULTS
    from concourse.bass_utils import run_bass_kernel_spmd

    x = np.ascontiguousarray(np.asarray(x, dtype=np.float32))
    s = float(np.asarray(scale, dtype=np.float32))
    B = x.shape[0]
    assert x.shape == (B, L) and B % N_CORES == 0
    rows = B // N_CORES

    h = _h_table(s)
    r = _pick_radius(x, s, h)
    nc = _build_program(rows, r, h, repeat=_repeat,
                        split_gpsimd=_split_gpsimd)

    shards = np.split(x, N_CORES, axis=0)
    in_maps = [{"x": sh} for sh in shards]
    res = run_bass_kernel_spmd(nc, in_maps, core_ids=list(range(N_CORES)))
    LAST_RESULTS = res
    return np.concatenate([res.results[i]["out"] for i in range(N_CORES)],
                          axis=0)
